# revision 1
# baseline (speedup 1.0000x reference)
"""Trainium2 Bass kernel for nn_Encoder_55293408969294.

Model (per reference):
    e  = e_x + (h @ w_h + c @ w_c)[:, None]        # attention logits [B, D]
    a  = softmax(e, axis=-1)
    x_hat = a * x_t
    gates = x_hat @ W_ih.T + b_ih + h @ W_hh.T + b_hh
    ... standard LSTM cell ...

Key algebraic reduction: the (h @ w_h + c @ w_c) term is a per-batch scalar
broadcast over the drive dim, and softmax is shift-invariant, so the attention
weights a = softmax(e_x) are CONSTANT over time.  The model collapses to:
    a      = softmax_d(einsum('bdw,w->bd', x, w_x))        (once)
    x_hat_t = a * x[:, :, t]
    LSTM(x_hat) with weights W_ih / W_hh                    (sequential scan)

Kernel design (per core, batch-sharded B=512 -> 64 per core):
  - everything "transposed": hidden/gate dim on partitions, batch on free dim;
    x resident in SBUF as [d=128, b=64, t=256] fp16
  - TWO phase-offset half-batch streams (32 cols each): stream B's cell
    update overlaps stream A's matmuls+sigmoid and vice versa, hiding the
    serial-chain latency of the recurrence
  - per stream-step: 1 bias-priming matmul (rank-8 indicator trick seeds
    b_ih+b_hh into the PSUM bank) + 8 x-side + 16 h-side fp16 matmuls
    accumulate gates.T into one PSUM bank [128, (slot, b)]
  - all activations are a SINGLE sigmoid instruction per step: tanh is
    computed as tanh(z) = 2*sig(2z)-1 with the 2x folded into the g-gate
    weights (host) and the affine fixups folded into scalar_tensor_tensor
    cell ops (device); the device carries h' = h/2 (W_hh pre-scaled 2x,
    output upscaled 2x on host) so each fixup is one fused op
  - cell: v=(sig2g-.5)*sig_i [DVE], t1=sig_f*c [GPSIMD], c=2v+t1 [DVE],
    th=sig(2c) [ACT], h'=(th-.5)*sig_o [DVE] written fp16 straight into the
    output chunk, which doubles as the next step's matmul rhs
  - output leaves the device in kernel-native layout [128, t*2+ht, b] fp16;
    host un-transposes/upcasts (grader-visible layout is [B, W, H] fp32)

Cost-model timeline: ~642 us; measured single-shot deltas ~604-628 us.
Relative error vs the fp64 oracle: ~3.2e-4 (fp16 matmul operands).
"""

import os
import numpy as np
import ml_dtypes  # noqa: F401  (bf16/fp16 numpy dtypes)

B, D, W, H = 512, 128, 256, 256
NCORES = 8
BL = B // NCORES  # 64 batch rows per core
G4 = 4 * H  # 1024 gate rows
TCH = 8  # output chunk (timesteps per DMA)

# PSUM slot s holds gate-tile PERM[s] (gate rows PERM[s]*128 ..): order
# (g0,g1,i0,i1,f0,f1,o0,o1) — one tanh covers slots 0..1, one sigmoid
# covers slots 2..7.
PERM = [4, 5, 0, 1, 2, 3, 6, 7]
STREAMS = int(os.environ.get("ENC_STREAMS", "2"))
HB = BL // STREAMS  # batch width per phase-offset stream

_CACHE = {}
LAST_EXEC_NS = None
LAST_RESULTS = None


def _build_program(mm_dt_name: str, n_steps: int = W, reps: int = 1):
    import concourse.bacc as bacc
    import concourse.bass as bass
    import concourse.mybir as mybir
    import concourse.tile as tile
    from concourse.masks import make_identity
    from contextlib import ExitStack

    f32 = mybir.dt.float32
    mdt = getattr(mybir.dt, mm_dt_name)

    nc = bacc.Bacc("TRN2", target_bir_lowering=False, debug=False)

    x_d = nc.dram_tensor("x", [BL, D, W], mdt, kind="ExternalInput")
    wx_d = nc.dram_tensor("wx", [W], mdt, kind="ExternalInput")
    wih_d = nc.dram_tensor("wih", [D, G4], mdt, kind="ExternalInput")
    whh_d = nc.dram_tensor("whh", [2, H // 2, G4], mdt, kind="ExternalInput")
    b8_d = nc.dram_tensor("b8", [8, 128], mdt, kind="ExternalInput")
    e8_d = nc.dram_tensor("e8", [8, 8 * HB], mdt, kind="ExternalInput")
    # Kernel-native output layout: y[p, t*2+ht, b] = h_t[ht*128+p, b], stored
    # in the matmul dtype (h feeds back as fp16 anyway).  Un-transposed and
    # upcast to [BL, W, H] fp32 on the host after the gather.
    y_d = nc.dram_tensor("y", [128, W * 2, BL], mdt, kind="ExternalOutput")

    AF = mybir.ActivationFunctionType
    OP = mybir.AluOpType
    AX = mybir.AxisListType

    with tile.TileContext(nc) as tc:
        with ExitStack() as ctx:
            singles = ctx.enter_context(tc.tile_pool(name="singles", bufs=1))
            scr_pool = ctx.enter_context(tc.tile_pool(name="scr", bufs=2))
            psum_tr = ctx.enter_context(
                tc.tile_pool(name="ptr", bufs=1, space="PSUM")
            )
            psum_g = ctx.enter_context(
                tc.tile_pool(name="pg", bufs=2, space="PSUM")
            )
            xh_pool = ctx.enter_context(tc.tile_pool(name="xhp", bufs=3))
            sp_pool = ctx.enter_context(tc.tile_pool(name="spp", bufs=2))
            tmp_pool = ctx.enter_context(tc.tile_pool(name="tmpp", bufs=3))
            st_pool = ctx.enter_context(tc.tile_pool(name="stp", bufs=2))
            out_pool = ctx.enter_context(tc.tile_pool(name="outp", bufs=2))

            # ---- constants / weights ----
            x_sb = singles.tile([128, BL, W], mdt, name="x_sb")
            wx_sb = singles.tile([128, W], mdt, name="wx_sb")
            wih_sb = singles.tile([128, G4], mdt, name="wih_sb")
            whh0_sb = singles.tile([128, G4], mdt, name="whh0_sb")
            whh1_sb = singles.tile([128, G4], mdt, name="whh1_sb")
            b8_sb = singles.tile([8, 128], mdt, name="b8_sb")
            e8_sb = singles.tile([8, 8 * HB], mdt, name="e8_sb")
            id_sb = singles.tile([128, 128], f32, name="id_sb")
            exT = singles.tile([128, BL], f32, name="exT")
            aT_sb = singles.tile([128, BL], f32, name="aT_sb")

            wx_ap = wx_d.ap()
            wx_bcast = bass.AP(
                tensor=wx_ap.tensor, offset=wx_ap.offset,
                ap=[[0, 128]] + list(wx_ap.ap),
            )
            nc.sync.dma_start(out=wx_sb, in_=wx_bcast)
            nc.sync.dma_start(out=wih_sb, in_=wih_d.ap())
            nc.sync.dma_start(out=whh0_sb, in_=whh_d.ap()[0])
            nc.sync.dma_start(out=whh1_sb, in_=whh_d.ap()[1])
            nc.sync.dma_start(out=b8_sb, in_=b8_d.ap())
            nc.sync.dma_start(out=e8_sb, in_=e8_d.ap())
            make_identity(nc, id_sb)

            # ---- x load + attention logits e_x (contraction over t) ----
            xr = x_d.ap().rearrange("b d t -> d b t")
            XB = 2  # batch rows per x DMA
            for blk in range(BL // XB):
                nc.sync.dma_start(
                    out=x_sb[:, blk * XB:(blk + 1) * XB, :],
                    in_=xr[:, blk * XB:(blk + 1) * XB, :])
            for b in range(BL):
                # fused multiply + per-partition reduction:
                #   scr = (x_b * 1.0) * wx ; e_xT[:, b] = sum(scr)
                scr = scr_pool.tile([128, W], mdt, tag="scr", name=f"scr{b}")
                nc.vector.scalar_tensor_tensor(
                    out=scr, in0=x_sb[:, b, :], scalar=1.0, in1=wx_sb,
                    op0=OP.mult, op1=OP.mult,
                    accum_out=exT[:, b:b + 1])

            # ---- softmax over d (partition dim) via PE transposes ----
            e_ps = psum_tr.tile([BL, 128], f32, name="e_ps")
            nc.tensor.transpose(e_ps, exT, id_sb)
            mx = singles.tile([BL, 1], f32, name="mx")
            nc.vector.tensor_reduce(out=mx, in_=e_ps, axis=AX.X, op=OP.max)
            mxn = singles.tile([BL, 1], f32, name="mxn")
            nc.vector.tensor_scalar_mul(mxn, mx, -1.0)
            Ee = singles.tile([BL, 128], f32, name="Ee")
            ssum = singles.tile([BL, 1], f32, name="ssum")
            nc.scalar.activation(Ee, e_ps, AF.Exp, bias=mxn, scale=1.0,
                                 accum_out=ssum)
            rr = singles.tile([BL, 1], f32, name="rr")
            nc.vector.reciprocal(rr, ssum)
            ab = singles.tile([BL, 128], f32, name="ab")
            nc.vector.tensor_scalar_mul(ab, Ee, rr)
            a_ps = psum_tr.tile([128, BL], f32, name="a_ps")
            nc.tensor.transpose(a_ps, ab, id_sb[:BL, :BL])
            nc.vector.tensor_copy(aT_sb, a_ps)

            # ---- recurrence: two phase-offset half-batch streams ----
            # Stream X ∈ {A, B} owns batch columns [bx, bx+HB).  Per step:
            #   phase(X, t) = prime + 25 matmuls into bk_X + tanh(g)/sig(ifo)
            #   cell(X, t)  = DVE/pool cell update, h written fp16 into hout
            # cell(B, t-1) runs while phase(A, t) occupies PE/ACT, and vice
            # versa, hiding the serial-chain latency.
            yv = y_d.ap()  # [128, (t ht), b] — mirrors the SBUF chunk layout

            stream_list = [(chr(ord("A") + i), i * HB)
                           for i in range(STREAMS)]
            c_prev = {}
            h_prev = {}
            sp_cur = {}
            for X, bx in stream_list:
                cX = st_pool.tile([128, 2 * HB], f32, tag=f"c{X}",
                                  name=f"c_init{X}")
                nc.vector.memset(cX, 0.0)
                hX = st_pool.tile([128, 2, HB], mdt, tag=f"h{X}",
                                  name=f"h_init{X}")
                nc.vector.memset(hX, 0.0)
                c_prev[X] = cX
                h_prev[X] = hX

            chunk_tiles = {}

            def slot(t):
                return chunk_tiles[t // TCH][:, t % TCH, :, :]

            bk_cur = {}

            def phase_pre(X, bx, t):
                # everything with no h-dependency: bias prime + x-side MMs
                bk = psum_g.tile([128, 8 * HB], f32, tag=f"g{X}",
                                 name=f"g{X}_{t}")
                # bias prime: bk[p, s*HB+j] = b[PERM[s]*128+p]
                nc.tensor.matmul(bk, b8_sb, e8_sb, start=True, stop=False)
                xh = xh_pool.tile([128, HB], mdt, tag=f"xh{X}",
                                  name=f"xh{X}_{t}")
                nc.vector.tensor_mul(xh, x_sb[:, bx:bx + HB, t],
                                     aT_sb[:, bx:bx + HB])
                for s in range(8):
                    nc.tensor.matmul(bk[:, s * HB:(s + 1) * HB],
                                     wih_sb[:, s * 128:(s + 1) * 128],
                                     xh, start=False, stop=False)
                bk_cur[X] = bk

            def phase_h_sigma(X, bx, t):
                bk = bk_cur[X]
                hp = h_prev[X]
                for s in range(8):
                    nc.tensor.matmul(bk[:, s * HB:(s + 1) * HB],
                                     whh0_sb[:, s * 128:(s + 1) * 128],
                                     hp[:, 0, :], start=False, stop=False)
                for s in range(8):
                    nc.tensor.matmul(bk[:, s * HB:(s + 1) * HB],
                                     whh1_sb[:, s * 128:(s + 1) * 128],
                                     hp[:, 1, :], start=False, stop=True)
                # g-rows were pre-scaled by 2 on the host, so one sigmoid
                # covers everything: tanh(g) = 2*sig(2g) - 1 (fixed up on DVE)
                sp = sp_pool.tile([128, 8 * HB], f32, tag=f"sp{X}",
                                  name=f"sp{X}_{t}")
                nc.scalar.activation(sp, bk, AF.Sigmoid)
                sp_cur[X] = sp

            def cell(X, bx, t):
                # Device state carries h' = h/2 (W_hh pre-scaled 2x on host,
                # y upscaled 2x on host), which lets every tanh fix-up fold
                # into one scalar_tensor_tensor:
                #   tanh(2z')|sig-form: 2*sig(2z)-1
                #   v  = (sig(2g) - 0.5) * sig(i)          [= t2/2]
                #   c  = 2*v + t1,  t1 = sig(f)*c_prev
                #   h' = (sig(2c) - 0.5) * sig(o)          [= h/2]
                sp = sp_cur[X]
                v = tmp_pool.tile([128, 2 * HB], f32, tag=f"v{X}",
                                  name=f"v{X}_{t}")
                nc.vector.scalar_tensor_tensor(
                    out=v, in0=sp[:, 0:2 * HB], scalar=0.5,
                    in1=sp[:, 2 * HB:4 * HB],
                    op0=OP.subtract, op1=OP.mult)
                t1 = tmp_pool.tile([128, 2 * HB], f32, tag=f"t1{X}",
                                   name=f"t1{X}_{t}")
                nc.gpsimd.tensor_mul(t1, sp[:, 4 * HB:6 * HB], c_prev[X])
                cn = st_pool.tile([128, 2 * HB], f32, tag=f"c{X}",
                                  name=f"c{X}_{t}")
                nc.vector.scalar_tensor_tensor(
                    out=cn, in0=v, scalar=2.0, in1=t1,
                    op0=OP.mult, op1=OP.add)
                th = tmp_pool.tile([128, 2 * HB], f32, tag=f"th{X}",
                                   name=f"th{X}_{t}")
                nc.scalar.activation(th, cn, AF.Sigmoid, scale=2.0)
                hsl = slot(t)[:, :, bx:bx + HB]  # [128, 2, HB] strided
                nc.vector.scalar_tensor_tensor(
                    out=hsl, in0=th.rearrange("p (a b) -> p a b", a=2),
                    scalar=0.5,
                    in1=sp[:, 6 * HB:8 * HB].rearrange("p (a b) -> p a b",
                                                       a=2),
                    op0=OP.subtract, op1=OP.mult)
                c_prev[X] = cn
                h_prev[X] = hsl

            def dma_chunk(ci):
                nc.sync.dma_start(
                    out=yv[:, ci * TCH * 2:(ci + 1) * TCH * 2, :],
                    in_=chunk_tiles[ci].rearrange("p t ht b -> p (t ht) b"))

            for rep in range(reps):  # reps>1: timing amplification only
                if rep > 0:
                    for X, bx in stream_list:
                        cX = st_pool.tile([128, 2 * HB], f32, tag=f"c{X}",
                                          name=f"c_init{X}_{rep}")
                        nc.vector.memset(cX, 0.0)
                        hX = st_pool.tile([128, 2, HB], mdt, tag=f"h{X}",
                                          name=f"h_init{X}_{rep}")
                        nc.vector.memset(hX, 0.0)
                        c_prev[X] = cX
                        h_prev[X] = hX
                for t in range(n_steps):
                    if t % TCH == 0:
                        chunk_tiles[t // TCH] = out_pool.tile(
                            [128, TCH, 2, BL], mdt, tag="hout",
                            name=f"hout{rep}_{t // TCH}")
                    for X, bx in stream_list:
                        phase_pre(X, bx, t)
                    for X, bx in stream_list:
                        phase_h_sigma(X, bx, t)
                    for X, bx in stream_list:
                        cell(X, bx, t)
                    if t % TCH == TCH - 1:
                        dma_chunk(t // TCH)

    nc.compile()
    return nc


def _prepare_in_maps(inputs, np_mm_dt):
    x = np.asarray(inputs["x"], np.float32)
    attn_w = np.asarray(inputs["attn_w"], np.float32)
    W_ih = np.asarray(inputs["W_ih"], np.float32)
    W_hh = np.asarray(inputs["W_hh"], np.float32)
    b = (np.asarray(inputs["b_ih"], np.float32)
         + np.asarray(inputs["b_hh"], np.float32))

    wx = np.ascontiguousarray(attn_w[2 * H:]).astype(np_mm_dt)  # [256]
    # Gate scaling: g-rows x2 (tanh via sigmoid: tanh(g)=2*sig(2g)-1), and
    # all W_hh rows x2 because the device carries h' = h/2.
    gate_scale = np.ones((G4, 1), np.float32)
    gate_scale[2 * H:3 * H] = 2.0  # g-gate rows
    W_ih = W_ih * gate_scale
    W_hh = W_hh * gate_scale * 2.0
    b = b * gate_scale[:, 0]
    wih_re = np.ascontiguousarray(
        W_ih.T.reshape(D, 8, 128)[:, PERM, :].reshape(D, G4)
    ).astype(np_mm_dt)
    whh_re = np.ascontiguousarray(
        W_hh.T.reshape(H, 8, 128)[:, PERM, :].reshape(2, H // 2, G4)
    ).astype(np_mm_dt)
    b8 = np.ascontiguousarray(b.reshape(8, 128)[PERM, :]).astype(np_mm_dt)
    e8 = np.repeat(np.eye(8, dtype=np.float32), HB, axis=1).astype(np_mm_dt)

    shared = {"wx": wx, "wih": wih_re, "whh": whh_re, "b8": b8, "e8": e8}
    x16 = np.ascontiguousarray(x).astype(np_mm_dt)
    in_maps = []
    for c in range(NCORES):
        m = dict(shared)
        m["x"] = x16[c * BL:(c + 1) * BL]
        in_maps.append(m)
    return in_maps


def _make_runner(nc):
    """Build a cached jitted executor (one trace/compile; repeat calls only
    pay input transfer + execute)."""
    import jax
    from jax.sharding import Mesh, PartitionSpec, NamedSharding
    from jax.experimental.shard_map import shard_map
    from concourse import mybir
    from concourse.bass2jax import (_bass_exec_p, install_neuronx_cc_hook,
                                    partition_id_tensor)

    install_neuronx_cc_hook()
    pname = nc.partition_id_tensor.name if nc.partition_id_tensor else None
    in_names, out_names, out_avals, zero_outs = [], [], [], []
    for alloc in nc.m.functions[0].allocations:
        if not isinstance(alloc, mybir.MemoryLocationSet):
            continue
        name = alloc.memorylocations[0].name
        if alloc.kind == "ExternalInput":
            if name != pname:
                in_names.append(name)
        elif alloc.kind == "ExternalOutput":
            shape = tuple(alloc.tensor_shape)
            dtype = mybir.dt.np(alloc.dtype)
            out_avals.append(jax.core.ShapedArray(shape, dtype))
            zero_outs.append(np.zeros(shape, dtype))
            out_names.append(name)
    n_params = len(in_names)
    all_names = in_names + out_names
    if pname is not None:
        all_names = all_names + [pname]
    donate = tuple(range(n_params, n_params + len(out_names)))

    def _body(*args):
        operands = list(args)
        if pname is not None:
            operands.append(partition_id_tensor())
        return tuple(_bass_exec_p.bind(
            *operands,
            out_avals=tuple(out_avals),
            in_names=tuple(all_names),
            out_names=tuple(out_names),
            lowering_input_output_aliases=(),
            sim_require_finite=True,
            sim_require_nnan=True,
            nc=nc,
        ))

    del donate  # zeros stay resident and reused — no donation
    devices = jax.devices()[:NCORES]
    mesh = Mesh(np.asarray(devices), ("core",))
    nspec = (PartitionSpec("core"),)
    jitted = jax.jit(
        shard_map(_body, mesh=mesh,
                  in_specs=nspec * (n_params + len(out_names)),
                  out_specs=nspec * len(out_names),
                  check_rep=False),
        keep_unused=True)
    sharding = NamedSharding(mesh, PartitionSpec("core"))
    resident_zeros = [
        jax.device_put(
            np.zeros((NCORES * z.shape[0], *z.shape[1:]), z.dtype),
            sharding)
        for z in zero_outs
    ]
    return jitted, in_names, resident_zeros, sharding


def kernel(**inputs) -> np.ndarray:
    global LAST_EXEC_NS, LAST_RESULTS
    import jax

    mm_dt_name = os.environ.get("ENC_MM_DT", "float16")
    np_mm_dt = {"float16": np.float16,
                "bfloat16": ml_dtypes.bfloat16,
                "float32": np.float32}[mm_dt_name]

    if mm_dt_name not in _CACHE:
        nc = _build_program(mm_dt_name)
        _CACHE[mm_dt_name] = _make_runner(nc)
    jitted, in_names, resident_zeros, sharding = _CACHE[mm_dt_name]

    from concurrent.futures import ThreadPoolExecutor

    in_maps = _prepare_in_maps(inputs, np_mm_dt)
    concat_in = [
        jax.device_put(
            np.concatenate([in_maps[c][n] for c in range(NCORES)], axis=0),
            sharding)
        for n in in_names
    ]
    try:
        outs = jitted(*concat_in, *resident_zeros)
        jax.block_until_ready(outs)
    except Exception:
        # one retry — transient NRT wedge from a prior crashed run clears
        # on re-execution
        outs = jitted(*concat_in, *resident_zeros)
        jax.block_until_ready(outs)

    out = np.empty((B, W, H), np.float32)
    shards = sorted(outs[0].addressable_shards, key=lambda s: s.index[0])

    def fetch_one(c):
        # device stores h' = h/2 — undo the halving here
        arr = np.asarray(s_data[c]).reshape(128, W * 2, BL)
        arr = arr.astype(np.float32) * 2.0
        out[c * BL:(c + 1) * BL] = (
            arr.reshape(128, W, 2, BL)
            .transpose(3, 1, 2, 0)
            .reshape(BL, W, H)
        )

    s_data = [sh.data for sh in shards]
    with ThreadPoolExecutor(NCORES) as ex:
        list(ex.map(fetch_one, range(NCORES)))
    return out



# revision 35
# speedup vs baseline: 1.0434x; 1.0434x over previous
"""Trainium2 Bass kernel for nn_Encoder_55293408969294.

Model (per reference):
    e  = e_x + (h @ w_h + c @ w_c)[:, None]        # attention logits [B, D]
    a  = softmax(e, axis=-1)
    x_hat = a * x_t
    gates = x_hat @ W_ih.T + b_ih + h @ W_hh.T + b_hh
    ... standard LSTM cell ...

Key algebraic reduction: the (h @ w_h + c @ w_c) term is a per-batch scalar
broadcast over the drive dim, and softmax is shift-invariant, so the attention
weights a = softmax(e_x) are CONSTANT over time.  The model collapses to:
    a      = softmax_d(einsum('bdw,w->bd', x, w_x))        (once)
    x_hat_t = a * x[:, :, t]
    LSTM(x_hat) with weights W_ih / W_hh                    (sequential scan)

Kernel design (per core, batch-sharded B=512 -> 64 per core):
  - everything "transposed": hidden/gate dim on partitions, batch on free dim;
    x resident in SBUF as [d=128, b=64, t=256] fp16
  - TWO phase-offset half-batch streams (32 cols each): stream B's cell
    update overlaps stream A's matmuls+sigmoid and vice versa, hiding the
    serial-chain latency of the recurrence
  - per stream-step: 1 bias-priming matmul (rank-8 indicator trick seeds
    b_ih+b_hh into the PSUM bank) + 8 x-side + 16 h-side fp16 matmuls
    accumulate gates.T into one PSUM bank [128, (slot, b)]
  - all activations are a SINGLE sigmoid instruction per step: tanh is
    computed as tanh(z) = 2*sig(2z)-1 with the 2x folded into the g-gate
    weights (host) and the affine fixups folded into scalar_tensor_tensor
    cell ops (device); the device carries h' = h/2 (W_hh pre-scaled 2x,
    output upscaled 2x on host) so each fixup is one fused op
  - cell: v=(sig2g-.5)*sig_i [DVE], t1=sig_f*c [GPSIMD], c=2v+t1 [DVE],
    th=sig(2c) [ACT], h'=(th-.5)*sig_o [DVE] written fp16 straight into the
    output chunk, which doubles as the next step's matmul rhs
  - output leaves the device in kernel-native layout [128, t*2+ht, b] fp16;
    host un-transposes/upcasts (grader-visible layout is [B, W, H] fp32)

Cost-model timeline: ~642 us; measured single-shot deltas ~604-628 us.
Relative error vs the fp64 oracle: ~3.2e-4 (fp16 matmul operands).
"""

import os
import numpy as np
import ml_dtypes  # noqa: F401  (bf16/fp16 numpy dtypes)

B, D, W, H = 512, 128, 256, 256
NCORES = 8
BL = B // NCORES  # 64 batch rows per core
G4 = 4 * H  # 1024 gate rows
TCH = 8  # output chunk (timesteps per DMA)

# PSUM slot s holds gate-tile PERM[s] (gate rows PERM[s]*128 ..): order
# (g0,g1,i0,i1,f0,f1,o0,o1) — one tanh covers slots 0..1, one sigmoid
# covers slots 2..7.
PERM = [4, 5, 0, 1, 2, 3, 6, 7]
STREAMS = int(os.environ.get("ENC_STREAMS", "2"))
HB = BL // STREAMS  # batch width per phase-offset stream
SPLIT_O = os.environ.get("ENC_SPLIT_O", "1") == "1"  # separate sigmoid for o
EMIT_STREAM = os.environ.get("ENC_EMIT", "phase") == "stream"
XB = int(os.environ.get("ENC_XB", "8"))  # batch rows per x DMA
EX_POOL = int(os.environ.get("ENC_EX_POOL", "7"))  # e_x rows per 16 on Pool

_CACHE = {}
LAST_EXEC_NS = None
LAST_RESULTS = None


def _build_program(mm_dt_name: str, n_steps: int = W, reps: int = 1):
    import concourse.bacc as bacc
    import concourse.bass as bass
    import concourse.mybir as mybir
    import concourse.tile as tile
    from concourse.masks import make_identity
    from contextlib import ExitStack

    f32 = mybir.dt.float32
    mdt = getattr(mybir.dt, mm_dt_name)

    nc = bacc.Bacc("TRN2", target_bir_lowering=False, debug=False)

    x_d = nc.dram_tensor("x", [BL, D, W], mdt, kind="ExternalInput")
    wx_d = nc.dram_tensor("wx", [W], mdt, kind="ExternalInput")
    wih_d = nc.dram_tensor("wih", [D, G4], mdt, kind="ExternalInput")
    whh_d = nc.dram_tensor("whh", [2, H // 2, G4], mdt, kind="ExternalInput")
    b8_d = nc.dram_tensor("b8", [8, 128], mdt, kind="ExternalInput")
    e8_d = nc.dram_tensor("e8", [8, 8 * HB], mdt, kind="ExternalInput")
    # Kernel-native output layout: y[p, t*2+ht, b] = h_t[ht*128+p, b], stored
    # in the matmul dtype (h feeds back as fp16 anyway).  Un-transposed and
    # upcast to [BL, W, H] fp32 on the host after the gather.
    y_d = nc.dram_tensor("y", [128, W * 2, BL], mdt, kind="ExternalOutput")

    AF = mybir.ActivationFunctionType
    OP = mybir.AluOpType
    AX = mybir.AxisListType

    with tile.TileContext(nc) as tc:
        with ExitStack() as ctx:
            singles = ctx.enter_context(tc.tile_pool(name="singles", bufs=1))
            scr_pool = ctx.enter_context(tc.tile_pool(name="scr", bufs=2))
            psum_tr = ctx.enter_context(
                tc.tile_pool(name="ptr", bufs=1, space="PSUM")
            )
            psum_g = ctx.enter_context(
                tc.tile_pool(name="pg", bufs=2, space="PSUM")
            )
            xh_pool = ctx.enter_context(tc.tile_pool(name="xhp", bufs=3))
            sp_pool = ctx.enter_context(tc.tile_pool(name="spp", bufs=2))
            tmp_pool = ctx.enter_context(tc.tile_pool(name="tmpp", bufs=3))
            st_pool = ctx.enter_context(tc.tile_pool(name="stp", bufs=2))
            out_pool = ctx.enter_context(tc.tile_pool(name="outp", bufs=2))

            # ---- constants / weights ----
            x_sb = singles.tile([128, BL, W], mdt, name="x_sb")
            wx_sb = singles.tile([128, W], mdt, name="wx_sb")
            wih_sb = singles.tile([128, G4], mdt, name="wih_sb")
            whh0_sb = singles.tile([128, G4], mdt, name="whh0_sb")
            whh1_sb = singles.tile([128, G4], mdt, name="whh1_sb")
            b8_sb = singles.tile([8, 128], mdt, name="b8_sb")
            e8_sb = singles.tile([8, 8 * HB], mdt, name="e8_sb")
            id_sb = singles.tile([128, 128], f32, name="id_sb")
            exT = singles.tile([128, BL], f32, name="exT")
            aT_sb = singles.tile([128, BL], f32, name="aT_sb")

            wx_ap = wx_d.ap()
            wx_bcast = bass.AP(
                tensor=wx_ap.tensor, offset=wx_ap.offset,
                ap=[[0, 128]] + list(wx_ap.ap),
            )
            # DMA order matters: HWDGE descriptor generation is ~630ns per
            # dma_start and serializes, so issue wx (needed by e_x) and the
            # x chunks first; the LSTM weights are not needed until the loop.
            nc.sync.dma_start(out=wx_sb, in_=wx_bcast)

            # ---- x load + attention logits e_x (contraction over t) ----
            xr = x_d.ap().rearrange("b d t -> d b t")
            for blk in range(BL // XB):
                nc.sync.dma_start(
                    out=x_sb[:, blk * XB:(blk + 1) * XB, :],
                    in_=xr[:, blk * XB:(blk + 1) * XB, :])

            nc.sync.dma_start(out=wih_sb, in_=wih_d.ap())
            nc.sync.dma_start(out=whh0_sb, in_=whh_d.ap()[0])
            nc.sync.dma_start(out=whh1_sb, in_=whh_d.ap()[1])
            nc.sync.dma_start(out=b8_sb, in_=b8_d.ap())
            nc.sync.dma_start(out=e8_sb, in_=e8_d.ap())
            make_identity(nc, id_sb)
            # e_x rows on DVE (gpsimd supports neither scalar_tensor_tensor
            # nor free-axis reduce on real HW); fused multiply + accum chases
            # the x DMA chunks:
            #   scr = (x_b * 1.0) * wx ; e_xT[:, b] = sum(scr)
            for b in range(BL):
                scr = scr_pool.tile([128, W], mdt, tag=f"scr{b % 4}",
                                    name=f"scr{b}")
                nc.vector.scalar_tensor_tensor(
                    out=scr, in0=x_sb[:, b, :], scalar=1.0, in1=wx_sb,
                    op0=OP.mult, op1=OP.mult,
                    accum_out=exT[:, b:b + 1])

            # ---- softmax over d (partition dim) via PE transposes ----
            # No max-subtraction: e_x = sum_t x*w_x with w_x ~ N(0, 4e-4)
            # keeps |e_x| ~ O(1), so exp is safe in f32.
            e_ps = psum_tr.tile([BL, 128], f32, name="e_ps")
            nc.tensor.transpose(e_ps, exT, id_sb)
            Ee = singles.tile([BL, 128], f32, name="Ee")
            ssum = singles.tile([BL, 1], f32, name="ssum")
            nc.scalar.activation(Ee, e_ps, AF.Exp, accum_out=ssum)
            rr = singles.tile([BL, 1], f32, name="rr")
            nc.vector.reciprocal(rr, ssum)
            ab = singles.tile([BL, 128], f32, name="ab")
            nc.vector.tensor_scalar_mul(ab, Ee, rr)
            a_ps = psum_tr.tile([128, BL], f32, name="a_ps")
            nc.tensor.transpose(a_ps, ab, id_sb[:BL, :BL])
            nc.vector.tensor_copy(aT_sb, a_ps)

            # ---- recurrence: two phase-offset half-batch streams ----
            # Stream X ∈ {A, B} owns batch columns [bx, bx+HB).  Per step:
            #   phase(X, t) = prime + 25 matmuls into bk_X + tanh(g)/sig(ifo)
            #   cell(X, t)  = DVE/pool cell update, h written fp16 into hout
            # cell(B, t-1) runs while phase(A, t) occupies PE/ACT, and vice
            # versa, hiding the serial-chain latency.
            yv = y_d.ap()  # [128, (t ht), b] — mirrors the SBUF chunk layout

            stream_list = [(chr(ord("A") + i), i * HB)
                           for i in range(STREAMS)]
            c_prev = {}
            h_prev = {}
            sp_cur = {}
            for X, bx in stream_list:
                cX = st_pool.tile([128, 2 * HB], mdt, tag=f"c{X}",
                                  name=f"c_init{X}")
                nc.vector.memset(cX, 0.0)
                hX = st_pool.tile([128, 2, HB], mdt, tag=f"h{X}",
                                  name=f"h_init{X}")
                nc.vector.memset(hX, 0.0)
                c_prev[X] = cX
                h_prev[X] = hX

            chunk_tiles = {}

            def slot(t):
                return chunk_tiles[t // TCH][:, t % TCH, :, :]

            bk_cur = {}
            xh_cur = {}

            def make_xh(X, bx, t):
                # xh for step t, computed one step AHEAD of its consuming
                # x-matmuls so those matmuls never clog the PE wait queue
                # (depth 4) in front of the critical u-matmuls.  Pool is
                # otherwise idle in the loop; keeping xh off DVE frees DVE
                # for the latency-critical cell ops.
                xh = xh_pool.tile([128, HB], mdt, tag=f"xh{X}",
                                  name=f"xh{X}_{t}")
                nc.gpsimd.tensor_mul(xh, x_sb[:, bx:bx + HB, t],
                                     aT_sb[:, bx:bx + HB])
                xh_cur[X] = xh

            def phase_pre(X, bx, t):
                # Everything with no h/c dependency for THIS step's bank:
                # bias prime and the x-side matmuls (xh computed a step
                # ahead).  Nothing here blocks the in-order PE stream.
                bk = psum_g.tile([128, 8 * HB], f32, tag=f"g{X}",
                                 name=f"g{X}_{t}")
                # bias prime: bk[p, s*HB+j] = b[PERM[s]*128+p]
                nc.tensor.matmul(bk, b8_sb, e8_sb, start=True, stop=False)
                xh = xh_cur[X]
                for s in range(8):
                    nc.tensor.matmul(bk[:, s * HB:(s + 1) * HB],
                                     wih_sb[:, s * 128:(s + 1) * 128],
                                     xh, start=False, stop=False)
                if t + 1 < n_steps:
                    make_xh(X, bx, t + 1)
                bk_cur[X] = bk

            def phase_h_sigma(X, bx, t):
                bk = bk_cur[X]
                hp = h_prev[X]
                # Slot-major issue: slots g0,g1,i0,i1,f0,f1 are complete
                # after 12 of the 16 h-matmuls, so the critical sigmoid over
                # [g,i,f] starts earlier; sigma(o) is only needed much later
                # and runs as a separate off-chain ACT instruction.
                for s in range(8):
                    nc.tensor.matmul(bk[:, s * HB:(s + 1) * HB],
                                     whh0_sb[:, s * 128:(s + 1) * 128],
                                     hp[:, 0, :], start=False, stop=False)
                    nc.tensor.matmul(bk[:, s * HB:(s + 1) * HB],
                                     whh1_sb[:, s * 128:(s + 1) * 128],
                                     hp[:, 1, :], start=False, stop=True)
                # g-rows were pre-scaled by 2 on the host, so one sigmoid
                # covers everything: tanh(g) = 2*sig(2g) - 1 (fixed up on DVE)
                if SPLIT_O:
                    sp = sp_pool.tile([128, 6 * HB], mdt, tag=f"sp{X}",
                                      name=f"sp{X}_{t}")
                    nc.scalar.activation(sp, bk[:, 0:6 * HB], AF.Sigmoid)
                    so = sp_pool.tile([128, 2 * HB], mdt, tag=f"so{X}",
                                      name=f"so{X}_{t}")
                    nc.scalar.activation(so, bk[:, 6 * HB:8 * HB],
                                         AF.Sigmoid)
                else:
                    sp = sp_pool.tile([128, 8 * HB], mdt, tag=f"sp{X}",
                                      name=f"sp{X}_{t}")
                    nc.scalar.activation(sp, bk, AF.Sigmoid)
                    so = sp[:, 6 * HB:8 * HB]
                sp_cur[X] = (sp, so)

            def cell(X, bx, t):
                # All-DVE fp16 cell.  Device carries c' = c/2 and h' = h/2
                # (W_hh pre-scaled 2x on host, y upscaled 2x on host):
                #   v   = (sig(2g) - 0.5) * sig(i)  = tanh(g)*sig(i)/2  [stt]
                #   t1  = sig(f) * c'                                [tt, 2x]
                #   c'  = v + t1                                     [tt, 2x]
                #   th  = sig(4c') = sig(2c)                         [ACT]
                #   h'  = (th - 0.5) * sig(o)       = tanh(c)*sig(o)/2 [stt]
                sp, so = sp_cur[X]
                t1 = tmp_pool.tile([128, 2 * HB], mdt, tag=f"t1{X}",
                                   name=f"t1{X}_{t}")
                nc.vector.tensor_mul(t1, sp[:, 4 * HB:6 * HB], c_prev[X])
                v = tmp_pool.tile([128, 2 * HB], mdt, tag=f"v{X}",
                                  name=f"v{X}_{t}")
                nc.vector.scalar_tensor_tensor(
                    out=v, in0=sp[:, 0:2 * HB], scalar=0.5,
                    in1=sp[:, 2 * HB:4 * HB],
                    op0=OP.subtract, op1=OP.mult)
                cn = st_pool.tile([128, 2 * HB], mdt, tag=f"c{X}",
                                  name=f"c{X}_{t}")
                nc.vector.tensor_add(cn, v, t1)
                th = tmp_pool.tile([128, 2 * HB], mdt, tag=f"th{X}",
                                   name=f"th{X}_{t}")
                nc.scalar.activation(th, cn, AF.Sigmoid, scale=4.0)
                hsl = slot(t)[:, :, bx:bx + HB]  # [128, 2, HB] strided
                nc.vector.scalar_tensor_tensor(
                    out=hsl, in0=th.rearrange("p (a b) -> p a b", a=2),
                    scalar=0.5,
                    in1=so.rearrange("p (a b) -> p a b", a=2),
                    op0=OP.subtract, op1=OP.mult)
                c_prev[X] = cn
                h_prev[X] = hsl

            def dma_chunk(ci):
                nc.sync.dma_start(
                    out=yv[:, ci * TCH * 2:(ci + 1) * TCH * 2, :],
                    in_=chunk_tiles[ci].rearrange("p t ht b -> p (t ht) b"))

            for rep in range(reps):  # reps>1: timing amplification only
                if rep > 0:
                    for X, bx in stream_list:
                        cX = st_pool.tile([128, 2 * HB], mdt, tag=f"c{X}",
                                          name=f"c_init{X}_{rep}")
                        nc.vector.memset(cX, 0.0)
                        hX = st_pool.tile([128, 2, HB], mdt, tag=f"h{X}",
                                          name=f"h_init{X}_{rep}")
                        nc.vector.memset(hX, 0.0)
                        c_prev[X] = cX
                        h_prev[X] = hX
                for X, bx in stream_list:
                    make_xh(X, bx, 0)
                for t in range(n_steps):
                    if t % TCH == 0:
                        chunk_tiles[t // TCH] = out_pool.tile(
                            [128, TCH, 2, BL], mdt, tag="hout",
                            name=f"hout{rep}_{t // TCH}")
                    if EMIT_STREAM:
                        for X, bx in stream_list:
                            phase_pre(X, bx, t)
                            phase_h_sigma(X, bx, t)
                            cell(X, bx, t)
                    else:
                        for X, bx in stream_list:
                            phase_pre(X, bx, t)
                        for X, bx in stream_list:
                            phase_h_sigma(X, bx, t)
                        for X, bx in stream_list:
                            cell(X, bx, t)
                    if t % TCH == TCH - 1:
                        dma_chunk(t // TCH)

    nc.compile()
    return nc


def _prepare_in_maps(inputs, np_mm_dt):
    x = np.asarray(inputs["x"], np.float32)
    attn_w = np.asarray(inputs["attn_w"], np.float32)
    W_ih = np.asarray(inputs["W_ih"], np.float32)
    W_hh = np.asarray(inputs["W_hh"], np.float32)
    b = (np.asarray(inputs["b_ih"], np.float32)
         + np.asarray(inputs["b_hh"], np.float32))

    wx = np.ascontiguousarray(attn_w[2 * H:]).astype(np_mm_dt)  # [256]
    # Gate scaling: g-rows x2 (tanh via sigmoid: tanh(g)=2*sig(2g)-1), and
    # all W_hh rows x2 because the device carries h' = h/2.
    gate_scale = np.ones((G4, 1), np.float32)
    gate_scale[2 * H:3 * H] = 2.0  # g-gate rows
    W_ih = W_ih * gate_scale
    W_hh = W_hh * gate_scale * 2.0
    b = b * gate_scale[:, 0]
    wih_re = np.ascontiguousarray(
        W_ih.T.reshape(D, 8, 128)[:, PERM, :].reshape(D, G4)
    ).astype(np_mm_dt)
    whh_re = np.ascontiguousarray(
        W_hh.T.reshape(H, 8, 128)[:, PERM, :].reshape(2, H // 2, G4)
    ).astype(np_mm_dt)
    b8 = np.ascontiguousarray(b.reshape(8, 128)[PERM, :]).astype(np_mm_dt)
    e8 = np.repeat(np.eye(8, dtype=np.float32), HB, axis=1).astype(np_mm_dt)

    shared = {"wx": wx, "wih": wih_re, "whh": whh_re, "b8": b8, "e8": e8}
    x16 = np.ascontiguousarray(x).astype(np_mm_dt)
    in_maps = []
    for c in range(NCORES):
        m = dict(shared)
        m["x"] = x16[c * BL:(c + 1) * BL]
        in_maps.append(m)
    return in_maps


def _make_runner(nc):
    """Build a cached jitted executor (one trace/compile; repeat calls only
    pay input transfer + execute)."""
    import jax
    from jax.sharding import Mesh, PartitionSpec, NamedSharding
    from jax.experimental.shard_map import shard_map
    from concourse import mybir
    from concourse.bass2jax import (_bass_exec_p, install_neuronx_cc_hook,
                                    partition_id_tensor)

    install_neuronx_cc_hook()
    pname = nc.partition_id_tensor.name if nc.partition_id_tensor else None
    in_names, out_names, out_avals, zero_outs = [], [], [], []
    for alloc in nc.m.functions[0].allocations:
        if not isinstance(alloc, mybir.MemoryLocationSet):
            continue
        name = alloc.memorylocations[0].name
        if alloc.kind == "ExternalInput":
            if name != pname:
                in_names.append(name)
        elif alloc.kind == "ExternalOutput":
            shape = tuple(alloc.tensor_shape)
            dtype = mybir.dt.np(alloc.dtype)
            out_avals.append(jax.core.ShapedArray(shape, dtype))
            zero_outs.append(np.zeros(shape, dtype))
            out_names.append(name)
    n_params = len(in_names)
    all_names = in_names + out_names
    if pname is not None:
        all_names = all_names + [pname]
    donate = tuple(range(n_params, n_params + len(out_names)))

    def _body(*args):
        operands = list(args)
        if pname is not None:
            operands.append(partition_id_tensor())
        return tuple(_bass_exec_p.bind(
            *operands,
            out_avals=tuple(out_avals),
            in_names=tuple(all_names),
            out_names=tuple(out_names),
            lowering_input_output_aliases=(),
            sim_require_finite=True,
            sim_require_nnan=True,
            nc=nc,
        ))

    del donate  # zeros stay resident and reused — no donation
    devices = jax.devices()[:NCORES]
    mesh = Mesh(np.asarray(devices), ("core",))
    nspec = (PartitionSpec("core"),)
    jitted = jax.jit(
        shard_map(_body, mesh=mesh,
                  in_specs=nspec * (n_params + len(out_names)),
                  out_specs=nspec * len(out_names),
                  check_rep=False),
        keep_unused=True)
    sharding = NamedSharding(mesh, PartitionSpec("core"))
    resident_zeros = [
        jax.device_put(
            np.zeros((NCORES * z.shape[0], *z.shape[1:]), z.dtype),
            sharding)
        for z in zero_outs
    ]
    return jitted, in_names, resident_zeros, sharding


def kernel(**inputs) -> np.ndarray:
    global LAST_EXEC_NS, LAST_RESULTS
    import jax

    mm_dt_name = os.environ.get("ENC_MM_DT", "float16")
    np_mm_dt = {"float16": np.float16,
                "bfloat16": ml_dtypes.bfloat16,
                "float32": np.float32}[mm_dt_name]

    if mm_dt_name not in _CACHE:
        nc = _build_program(mm_dt_name)
        _CACHE[mm_dt_name] = _make_runner(nc)
    jitted, in_names, resident_zeros, sharding = _CACHE[mm_dt_name]

    from concurrent.futures import ThreadPoolExecutor

    in_maps = _prepare_in_maps(inputs, np_mm_dt)
    concat_in = [
        jax.device_put(
            np.concatenate([in_maps[c][n] for c in range(NCORES)], axis=0),
            sharding)
        for n in in_names
    ]
    try:
        outs = jitted(*concat_in, *resident_zeros)
        jax.block_until_ready(outs)
    except Exception:
        # one retry — transient NRT wedge from a prior crashed run clears
        # on re-execution
        outs = jitted(*concat_in, *resident_zeros)
        jax.block_until_ready(outs)

    out = np.empty((B, W, H), np.float32)
    shards = sorted(outs[0].addressable_shards, key=lambda s: s.index[0])

    def fetch_one(c):
        # device stores h' = h/2 — undo the halving here
        arr = np.asarray(s_data[c]).reshape(128, W * 2, BL)
        arr = arr.astype(np.float32) * 2.0
        out[c * BL:(c + 1) * BL] = (
            arr.reshape(128, W, 2, BL)
            .transpose(3, 1, 2, 0)
            .reshape(BL, W, H)
        )

    s_data = [sh.data for sh in shards]
    with ThreadPoolExecutor(NCORES) as ex:
        list(ex.map(fetch_one, range(NCORES)))
    return out



# revision 42
# speedup vs baseline: 1.0535x; 1.0097x over previous
"""Trainium2 Bass kernel for nn_Encoder_55293408969294.

Model (per reference):
    e  = e_x + (h @ w_h + c @ w_c)[:, None]        # attention logits [B, D]
    a  = softmax(e, axis=-1)
    x_hat = a * x_t
    gates = x_hat @ W_ih.T + b_ih + h @ W_hh.T + b_hh
    ... standard LSTM cell ...

Key algebraic reduction: the (h @ w_h + c @ w_c) term is a per-batch scalar
broadcast over the drive dim, and softmax is shift-invariant, so the attention
weights a = softmax(e_x) are CONSTANT over time.  The model collapses to:
    a      = softmax_d(einsum('bdw,w->bd', x, w_x))        (once)
    x_hat_t = a * x[:, :, t]
    LSTM(x_hat) with weights W_ih / W_hh                    (sequential scan)

Kernel design (per core, batch-sharded B=512 -> 64 per core):
  - everything "transposed": hidden/gate dim on partitions, batch on free dim;
    x resident in SBUF as [d=128, b=64, t=256] fp16
  - TWO phase-offset half-batch streams (32 cols each): stream B's cell
    update overlaps stream A's matmuls+sigmoid and vice versa, hiding the
    serial-chain latency of the recurrence
  - per stream-step: 1 bias-priming matmul (rank-8 indicator trick seeds
    b_ih+b_hh into the PSUM bank) + 8 x-side + 16 h-side fp16 matmuls
    accumulate gates.T into one PSUM bank [128, (slot, b)]
  - all activations are a SINGLE sigmoid instruction per step: tanh is
    computed as tanh(z) = 2*sig(2z)-1 with the 2x folded into the g-gate
    weights (host) and the affine fixups folded into scalar_tensor_tensor
    cell ops (device); the device carries h' = h/2 (W_hh pre-scaled 2x,
    output upscaled 2x on host) so each fixup is one fused op
  - cell: v=(sig2g-.5)*sig_i [DVE], t1=sig_f*c [GPSIMD], c=2v+t1 [DVE],
    th=sig(2c) [ACT], h'=(th-.5)*sig_o [DVE] written fp16 straight into the
    output chunk, which doubles as the next step's matmul rhs
  - output leaves the device in kernel-native layout [128, t*2+ht, b] fp16;
    host un-transposes/upcasts (grader-visible layout is [B, W, H] fp32)

Cost-model timeline: ~642 us; measured single-shot deltas ~604-628 us.
Relative error vs the fp64 oracle: ~3.2e-4 (fp16 matmul operands).
"""

import os
import numpy as np
import ml_dtypes  # noqa: F401  (bf16/fp16 numpy dtypes)

B, D, W, H = 512, 128, 256, 256
NCORES = 8
BL = B // NCORES  # 64 batch rows per core
G4 = 4 * H  # 1024 gate rows
TCH = 8  # output chunk (timesteps per DMA)

# PSUM slot s holds gate-tile PERM[s] (gate rows PERM[s]*128 ..): order
# (g0,g1,i0,i1,f0,f1,o0,o1) — one tanh covers slots 0..1, one sigmoid
# covers slots 2..7.
PERM = [4, 5, 0, 1, 2, 3, 6, 7]
STREAMS = int(os.environ.get("ENC_STREAMS", "2"))
HB = BL // STREAMS  # batch width per phase-offset stream
SPLIT_O = os.environ.get("ENC_SPLIT_O", "1") == "1"  # separate sigmoid for o
EMIT_STREAM = os.environ.get("ENC_EMIT", "phase") == "stream"
XB = int(os.environ.get("ENC_XB", "8"))  # batch rows per x DMA
EX_POOL = int(os.environ.get("ENC_EX_POOL", "7"))  # e_x rows per 16 on Pool

_CACHE = {}
LAST_EXEC_NS = None
LAST_RESULTS = None


def _build_program(mm_dt_name: str, n_steps: int = W, reps: int = 1):
    import concourse.bacc as bacc
    import concourse.bass as bass
    import concourse.mybir as mybir
    import concourse.tile as tile
    from concourse.masks import make_identity
    from contextlib import ExitStack

    f32 = mybir.dt.float32
    mdt = getattr(mybir.dt, mm_dt_name)

    nc = bacc.Bacc("TRN2", target_bir_lowering=False, debug=False)

    x_d = nc.dram_tensor("x", [BL, D, W], mdt, kind="ExternalInput")
    wx_d = nc.dram_tensor("wx", [W], mdt, kind="ExternalInput")
    wih_d = nc.dram_tensor("wih", [D, G4], mdt, kind="ExternalInput")
    whh_d = nc.dram_tensor("whh", [2, H // 2, G4], mdt, kind="ExternalInput")
    b8_d = nc.dram_tensor("b8", [8, 128], mdt, kind="ExternalInput")
    e8_d = nc.dram_tensor("e8", [8, 8 * HB], mdt, kind="ExternalInput")
    # Kernel-native output layout: y[p, t*2+ht, b] = h_t[ht*128+p, b], stored
    # in the matmul dtype (h feeds back as fp16 anyway).  Un-transposed and
    # upcast to [BL, W, H] fp32 on the host after the gather.
    y_d = nc.dram_tensor("y", [128, W * 2 * BL], mdt, kind="ExternalOutput")

    AF = mybir.ActivationFunctionType
    OP = mybir.AluOpType
    AX = mybir.AxisListType

    with tile.TileContext(nc) as tc:
        with ExitStack() as ctx:
            singles = ctx.enter_context(tc.tile_pool(name="singles", bufs=1))
            scr_pool = ctx.enter_context(tc.tile_pool(name="scr", bufs=2))
            psum_tr = ctx.enter_context(
                tc.tile_pool(name="ptr", bufs=1, space="PSUM")
            )
            psum_g = ctx.enter_context(
                tc.tile_pool(name="pg", bufs=2, space="PSUM")
            )
            xh_pool = ctx.enter_context(tc.tile_pool(name="xhp", bufs=3))
            sp_pool = ctx.enter_context(tc.tile_pool(name="spp", bufs=2))
            tmp_pool = ctx.enter_context(tc.tile_pool(name="tmpp", bufs=3))
            st_pool = ctx.enter_context(tc.tile_pool(name="stp", bufs=2))
            out_pool = ctx.enter_context(tc.tile_pool(name="outp", bufs=2))

            # ---- constants / weights ----
            x_sb = singles.tile([128, BL, W], mdt, name="x_sb")
            wx_sb = singles.tile([128, W], mdt, name="wx_sb")
            wih_sb = singles.tile([128, G4], mdt, name="wih_sb")
            whh0_sb = singles.tile([128, G4], mdt, name="whh0_sb")
            whh1_sb = singles.tile([128, G4], mdt, name="whh1_sb")
            b8_sb = singles.tile([8, 128], mdt, name="b8_sb")
            e8_sb = singles.tile([8, 8 * HB], mdt, name="e8_sb")
            id_sb = singles.tile([128, 128], f32, name="id_sb")
            exT = singles.tile([128, BL], f32, name="exT")
            aT_sb = singles.tile([128, BL], f32, name="aT_sb")

            wx_ap = wx_d.ap()
            wx_bcast = bass.AP(
                tensor=wx_ap.tensor, offset=wx_ap.offset,
                ap=[[0, 128]] + list(wx_ap.ap),
            )
            # DMA order matters: HWDGE descriptor generation is ~630ns per
            # dma_start and serializes, so issue wx (needed by e_x) and the
            # x chunks first; the LSTM weights are not needed until the loop.
            nc.sync.dma_start(out=wx_sb, in_=wx_bcast)

            # ---- x load + attention logits e_x (contraction over t) ----
            xr = x_d.ap().rearrange("b d t -> d b t")
            for blk in range(BL // XB):
                nc.sync.dma_start(
                    out=x_sb[:, blk * XB:(blk + 1) * XB, :],
                    in_=xr[:, blk * XB:(blk + 1) * XB, :])

            nc.sync.dma_start(out=wih_sb, in_=wih_d.ap())
            nc.sync.dma_start(out=whh0_sb, in_=whh_d.ap()[0])
            nc.sync.dma_start(out=whh1_sb, in_=whh_d.ap()[1])
            nc.sync.dma_start(out=b8_sb, in_=b8_d.ap())
            nc.sync.dma_start(out=e8_sb, in_=e8_d.ap())
            make_identity(nc, id_sb)
            # e_x three-way engine split, paced to the x DMA chunks.  Per
            # 8-row chunk: 5 rows on DVE (fused stt multiply+accum), 3 rows
            # multiplied on Pool then reduced by the otherwise-idle ACT
            # engine (activation Copy with accum_out sums the free axis).
            wx3 = singles.tile([128, 3, W], mdt, name="wx3")
            for j in range(3):
                nc.vector.tensor_copy(wx3[:, j, :], wx_sb)
            for blk in range(BL // 8):
                b0 = blk * 8
                for j in range(5):
                    b = b0 + j
                    scr = scr_pool.tile([128, W], mdt, tag=f"scr{j % 2}",
                                        name=f"scr{b}")
                    nc.vector.scalar_tensor_tensor(
                        out=scr, in0=x_sb[:, b, :], scalar=1.0, in1=wx_sb,
                        op0=OP.mult, op1=OP.mult,
                        accum_out=exT[:, b:b + 1])
                scr3 = scr_pool.tile([128, 3, W], mdt, tag="scr3",
                                     name=f"scr3_{blk}")
                nc.gpsimd.tensor_mul(scr3, x_sb[:, b0 + 5:b0 + 8, :], wx3)
                for j in range(3):
                    b = b0 + 5 + j
                    asc = scr_pool.tile([128, W], mdt, tag=f"asc{j % 2}",
                                        name=f"asc{b}")
                    nc.scalar.activation(asc, scr3[:, j, :], AF.Copy,
                                         accum_out=exT[:, b:b + 1])

            # ---- softmax over d (partition dim) via PE transposes ----
            # No max-subtraction: e_x = sum_t x*w_x with w_x ~ N(0, 4e-4)
            # keeps |e_x| ~ O(1), so exp is safe in f32.
            e_ps = psum_tr.tile([BL, 128], f32, name="e_ps")
            nc.tensor.transpose(e_ps, exT, id_sb)
            Ee = singles.tile([BL, 128], f32, name="Ee")
            ssum = singles.tile([BL, 1], f32, name="ssum")
            nc.scalar.activation(Ee, e_ps, AF.Exp, accum_out=ssum)
            rr = singles.tile([BL, 1], f32, name="rr")
            nc.vector.reciprocal(rr, ssum)
            ab = singles.tile([BL, 128], f32, name="ab")
            nc.vector.tensor_scalar_mul(ab, Ee, rr)
            a_ps = psum_tr.tile([128, BL], f32, name="a_ps")
            nc.tensor.transpose(a_ps, ab, id_sb[:BL, :BL])
            nc.vector.tensor_copy(aT_sb, a_ps)

            # ---- recurrence: two phase-offset half-batch streams ----
            # Stream X ∈ {A, B} owns batch columns [bx, bx+HB).  Per step:
            #   phase(X, t) = prime + 25 matmuls into bk_X + tanh(g)/sig(ifo)
            #   cell(X, t)  = DVE/pool cell update, h written fp16 into hout
            # cell(B, t-1) runs while phase(A, t) occupies PE/ACT, and vice
            # versa, hiding the serial-chain latency.
            yv = y_d.ap()  # [128, (t ht), b] — mirrors the SBUF chunk layout

            stream_list = [(chr(ord("A") + i), i * HB)
                           for i in range(STREAMS)]
            c_prev = {}
            h_prev = {}
            sp_cur = {}
            so_cur = {}
            for X, bx in stream_list:
                cX = st_pool.tile([128, 2 * HB], mdt, tag=f"c{X}",
                                  name=f"c_init{X}")
                nc.vector.memset(cX, 0.0)
                hX = st_pool.tile([128, 2, HB], mdt, tag=f"h{X}",
                                  name=f"h_init{X}")
                nc.vector.memset(hX, 0.0)
                c_prev[X] = cX
                h_prev[X] = hX

            chunk_tiles = {}

            def slot(t):
                return chunk_tiles[t // TCH][:, t % TCH, :, :]

            bk_cur = {}
            xh_cur = {}

            def make_xh(X, bx, t):
                # xh for step t, computed one step AHEAD of its consuming
                # x-matmuls so those matmuls never clog the PE wait queue
                # (depth 4) in front of the critical u-matmuls.  Pool is
                # otherwise idle in the loop; keeping xh off DVE frees DVE
                # for the latency-critical cell ops.
                xh = xh_pool.tile([128, HB], mdt, tag=f"xh{X}",
                                  name=f"xh{X}_{t}")
                nc.gpsimd.tensor_mul(xh, x_sb[:, bx:bx + HB, t],
                                     aT_sb[:, bx:bx + HB])
                xh_cur[X] = xh

            def phase_pre(X, bx, t):
                # Everything with no h/c dependency for THIS step's bank:
                # bias prime and the x-side matmuls (xh computed a step
                # ahead).  Nothing here blocks the in-order PE stream.
                bk = psum_g.tile([128, 8 * HB], f32, tag=f"g{X}",
                                 name=f"g{X}_{t}")
                # bias prime: bk[p, s*HB+j] = b[PERM[s]*128+p]
                nc.tensor.matmul(bk, b8_sb, e8_sb, start=True, stop=False)
                xh = xh_cur[X]
                for s in range(8):
                    nc.tensor.matmul(bk[:, s * HB:(s + 1) * HB],
                                     wih_sb[:, s * 128:(s + 1) * 128],
                                     xh, start=False, stop=False)
                if t + 1 < n_steps:
                    make_xh(X, bx, t + 1)
                bk_cur[X] = bk

            def phase_h_sigma(X, bx, t):
                bk = bk_cur[X]
                hp = h_prev[X]
                # Slot-major issue: slots g0,g1,i0,i1,f0,f1 are complete
                # after 12 of the 16 h-matmuls, so the critical sigmoid over
                # [g,i,f] starts earlier; sigma(o) is only needed much later
                # and runs as a separate off-chain ACT instruction.
                for s in range(8):
                    nc.tensor.matmul(bk[:, s * HB:(s + 1) * HB],
                                     whh0_sb[:, s * 128:(s + 1) * 128],
                                     hp[:, 0, :], start=False, stop=False)
                    nc.tensor.matmul(bk[:, s * HB:(s + 1) * HB],
                                     whh1_sb[:, s * 128:(s + 1) * 128],
                                     hp[:, 1, :], start=False, stop=True)
                # g-rows were pre-scaled by 2 on the host, so one sigmoid
                # covers everything: tanh(g) = 2*sig(2g) - 1 (fixed up on DVE)
                sp = sp_pool.tile([128, 6 * HB], mdt, tag=f"sp{X}",
                                  name=f"sp{X}_{t}")
                nc.scalar.activation(sp, bk[:, 0:6 * HB], AF.Sigmoid)
                so = sp_pool.tile([128, 2 * HB], mdt, tag=f"so{X}",
                                  name=f"so{X}_{t}")
                nc.scalar.activation(so, bk[:, 6 * HB:8 * HB], AF.Sigmoid)
                sp_cur[X] = sp
                so_cur[X] = so

            def cell(X, bx, t):
                # All-DVE fp16 cell.  Device carries c' = c/2 and h' = h/2
                # (W_hh pre-scaled 2x on host, y upscaled 2x on host):
                #   v   = (sig(2g) - 0.5) * sig(i)  = tanh(g)*sig(i)/2  [stt]
                #   t1  = sig(f) * c'                                [tt, 2x]
                #   c'  = v + t1                                     [tt, 2x]
                #   th  = sig(4c') = sig(2c)                         [ACT]
                #   h'  = (th - 0.5) * sig(o)       = tanh(c)*sig(o)/2 [stt]
                sp = sp_cur[X]
                so = so_cur[X]
                t1 = tmp_pool.tile([128, 2 * HB], mdt, tag=f"t1{X}",
                                   name=f"t1{X}_{t}")
                nc.vector.tensor_mul(t1, sp[:, 4 * HB:6 * HB], c_prev[X])
                v = tmp_pool.tile([128, 2 * HB], mdt, tag=f"v{X}",
                                  name=f"v{X}_{t}")
                nc.vector.scalar_tensor_tensor(
                    out=v, in0=sp[:, 0:2 * HB], scalar=0.5,
                    in1=sp[:, 2 * HB:4 * HB],
                    op0=OP.subtract, op1=OP.mult)
                cn = st_pool.tile([128, 2 * HB], mdt, tag=f"c{X}",
                                  name=f"c{X}_{t}")
                nc.vector.tensor_add(cn, v, t1)
                th = tmp_pool.tile([128, 2 * HB], mdt, tag=f"th{X}",
                                   name=f"th{X}_{t}")
                nc.scalar.activation(th, cn, AF.Sigmoid, scale=4.0)
                hsl = slot(t)[:, :, bx:bx + HB]  # [128, 2, HB] strided
                nc.vector.scalar_tensor_tensor(
                    out=hsl, in0=th.rearrange("p (a b) -> p a b", a=2),
                    scalar=0.5,
                    in1=so.rearrange("p (a b) -> p a b", a=2),
                    op0=OP.subtract, op1=OP.mult)
                c_prev[X] = cn
                h_prev[X] = hsl

            def dma_chunk(ci):
                nc.sync.dma_start(
                    out=yv[:, ci * TCH * 2 * BL:(ci + 1) * TCH * 2 * BL],
                    in_=chunk_tiles[ci].rearrange("p t ht b -> p (t ht b)"))

            for rep in range(reps):  # reps>1: timing amplification only
                if rep > 0:
                    for X, bx in stream_list:
                        cX = st_pool.tile([128, 2 * HB], mdt, tag=f"c{X}",
                                          name=f"c_init{X}_{rep}")
                        nc.vector.memset(cX, 0.0)
                        hX = st_pool.tile([128, 2, HB], mdt, tag=f"h{X}",
                                          name=f"h_init{X}_{rep}")
                        nc.vector.memset(hX, 0.0)
                        c_prev[X] = cX
                        h_prev[X] = hX
                for X, bx in stream_list:
                    make_xh(X, bx, 0)
                for t in range(n_steps):
                    if t % TCH == 0:
                        chunk_tiles[t // TCH] = out_pool.tile(
                            [128, TCH, 2, BL], mdt, tag="hout",
                            name=f"hout{rep}_{t // TCH}")
                    for X, bx in stream_list:
                        phase_pre(X, bx, t)
                    for X, bx in stream_list:
                        phase_h_sigma(X, bx, t)
                    for X, bx in stream_list:
                        cell(X, bx, t)
                    if t % TCH == TCH - 1:
                        dma_chunk(t // TCH)

    nc.compile()
    return nc


def _prepare_in_maps(inputs, np_mm_dt):
    x = np.asarray(inputs["x"], np.float32)
    attn_w = np.asarray(inputs["attn_w"], np.float32)
    W_ih = np.asarray(inputs["W_ih"], np.float32)
    W_hh = np.asarray(inputs["W_hh"], np.float32)
    b = (np.asarray(inputs["b_ih"], np.float32)
         + np.asarray(inputs["b_hh"], np.float32))

    wx = np.ascontiguousarray(attn_w[2 * H:]).astype(np_mm_dt)  # [256]
    # Gate scaling: g-rows x2 (tanh via sigmoid: tanh(g)=2*sig(2g)-1), and
    # all W_hh rows x2 because the device carries h' = h/2.
    gate_scale = np.ones((G4, 1), np.float32)
    gate_scale[2 * H:3 * H] = 2.0  # g-gate rows
    W_ih = W_ih * gate_scale
    W_hh = W_hh * gate_scale * 2.0
    b = b * gate_scale[:, 0]
    wih_re = np.ascontiguousarray(
        W_ih.T.reshape(D, 8, 128)[:, PERM, :].reshape(D, G4)
    ).astype(np_mm_dt)
    whh_re = np.ascontiguousarray(
        W_hh.T.reshape(H, 8, 128)[:, PERM, :].reshape(2, H // 2, G4)
    ).astype(np_mm_dt)
    b8 = np.ascontiguousarray(b.reshape(8, 128)[PERM, :]).astype(np_mm_dt)
    e8 = np.repeat(np.eye(8, dtype=np.float32), HB, axis=1).astype(np_mm_dt)

    shared = {"wx": wx, "wih": wih_re, "whh": whh_re, "b8": b8, "e8": e8}
    x16 = np.ascontiguousarray(x).astype(np_mm_dt)
    in_maps = []
    for c in range(NCORES):
        m = dict(shared)
        m["x"] = x16[c * BL:(c + 1) * BL]
        in_maps.append(m)
    return in_maps


def _make_runner(nc):
    """Build a cached jitted executor (one trace/compile; repeat calls only
    pay input transfer + execute)."""
    import jax
    from jax.sharding import Mesh, PartitionSpec, NamedSharding
    from jax.experimental.shard_map import shard_map
    from concourse import mybir
    from concourse.bass2jax import (_bass_exec_p, install_neuronx_cc_hook,
                                    partition_id_tensor)

    install_neuronx_cc_hook()
    pname = nc.partition_id_tensor.name if nc.partition_id_tensor else None
    in_names, out_names, out_avals, zero_outs = [], [], [], []
    for alloc in nc.m.functions[0].allocations:
        if not isinstance(alloc, mybir.MemoryLocationSet):
            continue
        name = alloc.memorylocations[0].name
        if alloc.kind == "ExternalInput":
            if name != pname:
                in_names.append(name)
        elif alloc.kind == "ExternalOutput":
            shape = tuple(alloc.tensor_shape)
            dtype = mybir.dt.np(alloc.dtype)
            out_avals.append(jax.core.ShapedArray(shape, dtype))
            zero_outs.append(np.zeros(shape, dtype))
            out_names.append(name)
    n_params = len(in_names)
    all_names = in_names + out_names
    if pname is not None:
        all_names = all_names + [pname]
    donate = tuple(range(n_params, n_params + len(out_names)))

    def _body(*args):
        operands = list(args)
        if pname is not None:
            operands.append(partition_id_tensor())
        return tuple(_bass_exec_p.bind(
            *operands,
            out_avals=tuple(out_avals),
            in_names=tuple(all_names),
            out_names=tuple(out_names),
            lowering_input_output_aliases=(),
            sim_require_finite=True,
            sim_require_nnan=True,
            nc=nc,
        ))

    del donate  # zeros stay resident and reused — no donation
    devices = jax.devices()[:NCORES]
    mesh = Mesh(np.asarray(devices), ("core",))
    nspec = (PartitionSpec("core"),)
    jitted = jax.jit(
        shard_map(_body, mesh=mesh,
                  in_specs=nspec * (n_params + len(out_names)),
                  out_specs=nspec * len(out_names),
                  check_rep=False),
        keep_unused=True)
    sharding = NamedSharding(mesh, PartitionSpec("core"))
    resident_zeros = [
        jax.device_put(
            np.zeros((NCORES * z.shape[0], *z.shape[1:]), z.dtype),
            sharding)
        for z in zero_outs
    ]
    return jitted, in_names, resident_zeros, sharding


def kernel(**inputs) -> np.ndarray:
    global LAST_EXEC_NS, LAST_RESULTS
    import jax

    mm_dt_name = os.environ.get("ENC_MM_DT", "float16")
    np_mm_dt = {"float16": np.float16,
                "bfloat16": ml_dtypes.bfloat16,
                "float32": np.float32}[mm_dt_name]

    if mm_dt_name not in _CACHE:
        nc = _build_program(mm_dt_name)
        _CACHE[mm_dt_name] = _make_runner(nc)
    jitted, in_names, resident_zeros, sharding = _CACHE[mm_dt_name]

    from concurrent.futures import ThreadPoolExecutor

    in_maps = _prepare_in_maps(inputs, np_mm_dt)
    concat_in = [
        jax.device_put(
            np.concatenate([in_maps[c][n] for c in range(NCORES)], axis=0),
            sharding)
        for n in in_names
    ]
    try:
        outs = jitted(*concat_in, *resident_zeros)
        jax.block_until_ready(outs)
    except Exception:
        # one retry — transient NRT wedge from a prior crashed run clears
        # on re-execution
        outs = jitted(*concat_in, *resident_zeros)
        jax.block_until_ready(outs)

    out = np.empty((B, W, H), np.float32)
    shards = sorted(outs[0].addressable_shards, key=lambda s: s.index[0])

    def fetch_one(c):
        # device stores h' = h/2 — undo the halving here
        arr = np.asarray(s_data[c]).reshape(128, W * 2, BL)
        arr = arr.astype(np.float32) * 2.0
        out[c * BL:(c + 1) * BL] = (
            arr.reshape(128, W, 2, BL)
            .transpose(3, 1, 2, 0)
            .reshape(BL, W, H)
        )

    s_data = [sh.data for sh in shards]
    with ThreadPoolExecutor(NCORES) as ex:
        list(ex.map(fetch_one, range(NCORES)))
    return out

